# revision 1
# baseline (speedup 1.0000x reference)
"""Grouped multi-query attention (group axis summed) on 8 trn2 NeuronCores.

Math: reference sums the g axis of the grouped Q heads inside the score
einsum, so the whole module collapses to standard 8-head attention with
W_Qeff[n] = sum_g W_Q[4n+g] (and the 1/sqrt(64) score scale folded in).

Sharding: core c -> (batch b = c//2, kv-head half = c%2). Each core runs
4 heads of one batch and produces a full [2048, 2048] partial of the
output projection; the host sums the two halves per batch.

Per-core layout (all matmuls are out = lhsT.T @ rhs, bf16 in / f32 acc):
  xT [d, t] host-pretransposed; Q_T/K_T [2*64 head-pair rows, t] built by
  projection matmuls; V [t, 4*64] built directly; scores computed
  transposed S_T[k, q] = K @ Q^T so softmax/PV need no transposes at all;
  per-head Z lands in psum partitions 64*rj..64*rj+63 via tile_position
  col offsets; softmax denominators come from a packed M=1 ones-matmul,
  extracted across partitions with a tiny DMA, inverted, and broadcast
  back over 64 partitions with a K=1 ones-matmul. The two heads of a pair
  run concurrently in disjoint PE row/col groups (tile_position packing);
  causal masking is a multiplicative bf16 {0,1} DVE mult on the exp'd
  tile (4x mode); out-projection is interleaved per q-block round.
"""

import numpy as np

S = 2048
D = 2048
HD = 64
NKV = 8
GQ = 4  # grouped q heads per kv head (summed)
HPC = 4  # heads per core
TB = 512
QB = 512
NTB = S // TB
NDC = D // 128
NQB = S // QB
NKT = S // 128
IGNORE = -100000.0

_CACHE = {}


def _build_program():
    import concourse.bass as bass
    import concourse.tile as tile
    from concourse import bacc, mybir

    F32 = mybir.dt.float32
    BF16 = mybir.dt.bfloat16
    AF = mybir.ActivationFunctionType

    nc = bacc.Bacc("TRN2", target_bir_lowering=False, debug=False,
                   enable_asserts=False, num_devices=8)

    xT_d = nc.dram_tensor("xT", [D, S], BF16, kind="ExternalInput").ap()
    wq_d = nc.dram_tensor("wq", [D, HPC * HD], BF16, kind="ExternalInput").ap()
    wk_d = nc.dram_tensor("wk", [D, HPC * HD], BF16, kind="ExternalInput").ap()
    wv_d = nc.dram_tensor("wv", [D, HPC * HD], BF16, kind="ExternalInput").ap()
    wo_d = nc.dram_tensor("wo", [HPC * HD, D], BF16, kind="ExternalInput").ap()
    mask_d = nc.dram_tensor("mask", [128, 4, QB], BF16, kind="ExternalInput").ap()
    o_d = nc.dram_tensor("o", [S, D], F32, kind="ExternalOutput").ap()

    def r(ap):
        return ap

    import contextlib
    with tile.TileContext(nc) as tc, \
            nc.allow_low_precision(reason="bf16 matmul operands by design"):
        with (
            tc.tile_pool(name="singles", bufs=1) as singles,
            tc.tile_pool(name="persist", bufs=1) as persist,
            tc.tile_pool(name="work", bufs=4) as work,
            tc.tile_pool(name="outsb", bufs=3) as outsb,
            tc.tile_pool(name="tiny", bufs=4) as tiny,
            tc.tile_pool(name="bcsb", bufs=2) as bcsb,
        ):
            # constants + weights
            wq_sb = singles.tile([128, NDC, HPC * HD], BF16, tag="wq")
            wk_sb = singles.tile([128, NDC, HPC * HD], BF16, tag="wk")
            wv_sb = singles.tile([128, NDC, HPC * HD], BF16, tag="wv")
            wo_sb = singles.tile([128, 2, D], BF16, tag="wo")
            mask_sb = singles.tile([128, 4, QB], BF16, tag="mask")
            ones_col = singles.tile([128, 1], BF16, tag="onec")
            ones_row = singles.tile([1, HD], BF16, tag="oner")

            nc.sync.dma_start(out=wq_sb, in_=wq_d.rearrange("(c p) n -> p c n", p=128))
            nc.sync.dma_start(out=wk_sb, in_=wk_d.rearrange("(c p) n -> p c n", p=128))
            nc.sync.dma_start(out=wv_sb, in_=wv_d.rearrange("(c p) n -> p c n", p=128))
            nc.sync.dma_start(out=wo_sb, in_=wo_d.rearrange("(g p) d -> p g d", p=128))
            nc.sync.dma_start(out=mask_sb, in_=mask_d)
            nc.vector.memset(ones_col, 1.0)
            nc.vector.memset(ones_row, 1.0)

            qT_sb = persist.tile([128, 2, S], BF16, tag="qT")
            kT_sb = persist.tile([128, 2, S], BF16, tag="kT")
            v_sb = persist.tile([128, NKT, HPC * HD], BF16, tag="v")
            z_sb = persist.tile([128, 2, S], BF16, tag="z")

            # ---- phase 1: projections ----
            with tc.tile_pool(name="ph1ps", bufs=4,
                              space=bass.MemorySpace.PSUM) as ph1ps:
                for tb in range(NTB):
                    ps_q = [ph1ps.tile([128, TB], F32, tag="qk", name=f"psq{tb}_{i}") for i in range(2)]
                    ps_k = [ph1ps.tile([128, TB], F32, tag="qk", name=f"psk{tb}_{i}") for i in range(2)]
                    ps_v = [ph1ps.tile([128, HPC * HD], F32, tag="v", name=f"psv{tb}_{i}") for i in range(4)]
                    for dc in range(NDC):
                        xt = work.tile([128, TB], BF16, tag="xt")
                        nc.sync.dma_start(
                            out=xt,
                            in_=xT_d[dc * 128:(dc + 1) * 128, tb * TB:(tb + 1) * TB])
                        st = dict(start=(dc == 0), stop=(dc == NDC - 1))
                        for g in range(2):
                            nc.tensor.matmul(
                                ps_q[g], r(wq_sb[:, dc, 128 * g:128 * (g + 1)]),
                                r(xt), **st)
                            nc.tensor.matmul(
                                ps_k[g], r(wk_sb[:, dc, 128 * g:128 * (g + 1)]),
                                r(xt), **st)
                        for tt in range(4):
                            nc.tensor.matmul(
                                ps_v[tt], r(xt[:, tt * 128:(tt + 1) * 128]),
                                r(wv_sb[:, dc, :]), **st)
                    for g in range(2):
                        nc.scalar.copy(out=qT_sb[:, g, tb * TB:(tb + 1) * TB], in_=ps_q[g])
                        nc.scalar.copy(out=kT_sb[:, g, tb * TB:(tb + 1) * TB], in_=ps_k[g])
                    for tt in range(4):
                        nc.vector.tensor_copy(
                            out=v_sb[:, 4 * tb + tt, :], in_=ps_v[tt])

            # ---- phase 2: attention, phase 3: output projection ----
            with (
                tc.tile_pool(name="sps", bufs=2, space=bass.MemorySpace.PSUM) as sps,
                tc.tile_pool(name="zps", bufs=2, space=bass.MemorySpace.PSUM) as zps,
                tc.tile_pool(name="smps", bufs=1, space=bass.MemorySpace.PSUM) as smps,
                tc.tile_pool(name="bcps", bufs=1, space=bass.MemorySpace.PSUM) as bcps,
                tc.tile_pool(name="ops", bufs=2, space=bass.MemorySpace.PSUM) as ops,
            ):
                for jq in range(NQB):
                    nkt = 4 * (jq + 1)
                    for g in range(2):
                        # both heads of pair g packed into disjoint array
                        # regions: jj=0 -> z rows 64:128, sums row 32;
                        # jj=1 -> z rows 0:64, sums row 64
                        ps_z = zps.tile([128, QB], F32, tag="z",
                                        name=f"z{jq}_{g}")
                        ps_sm = smps.tile([128, QB], F32, tag="sm",
                                          name=f"sm{jq}_{g}")
                        for ik in range(nkt):
                            p2 = []
                            for jj in range(2):
                                ps_s = sps.tile([128, QB], F32, tag="s",
                                                name=f"s{jq}_{g}_{ik}_{jj}")
                                nc.tensor.matmul(
                                    ps_s,
                                    kT_sb[64 * jj:64 * (jj + 1), g,
                                          ik * 128:(ik + 1) * 128],
                                    qT_sb[64 * jj:64 * (jj + 1), g,
                                          jq * QB:(jq + 1) * QB],
                                    start=True, stop=True)
                                p_sb = work.tile([128, QB], BF16, tag="p",
                                                 name=f"p{jq}_{g}_{ik}_{jj}")
                                nc.scalar.activation(out=p_sb, in_=ps_s,
                                                     func=AF.Exp)
                                if ik >= 4 * jq:
                                    nc.vector.tensor_mul(
                                        p_sb, p_sb, mask_sb[:, ik - 4 * jq, :])
                                p2.append(p_sb)
                            st = dict(start=(ik == 0), stop=(ik == nkt - 1),
                                      skip_group_check=True)
                            for jj, p_sb in enumerate(p2):
                                rj = 1 - jj
                                nc.tensor.matmul(
                                    ps_z[64 * rj:64 * (rj + 1), :],
                                    v_sb[:, ik, HD * (2 * g + jj):
                                         HD * (2 * g + jj + 1)],
                                    p_sb, tile_position=(0, 64 * rj), **st)
                            for jj, p_sb in enumerate(p2):
                                sc = 32 if jj == 0 else 64
                                nc.tensor.matmul(
                                    ps_sm[sc:sc + 1, :], ones_col, p_sb,
                                    tile_position=(0, sc), **st)
                        for jj in range(2):
                            rj = 1 - jj
                            sc = 32 if jj == 0 else 64
                            sumhi = tiny.tile([65, QB], F32, tag="sumhi",
                                              name=f"sh{jq}_{g}_{jj}")
                            nc.scalar.copy(out=sumhi[sc:sc + 1, :],
                                           in_=ps_sm[sc:sc + 1, :])
                            sums_sb = tiny.tile([1, QB], F32, tag="sums",
                                                name=f"su{jq}_{g}_{jj}")
                            nc.gpsimd.dma_start(out=sums_sb,
                                                in_=sumhi[sc:sc + 1, :])
                            recip_sb = tiny.tile([1, QB], BF16, tag="recip",
                                                 name=f"re{jq}_{g}_{jj}")
                            nc.vector.reciprocal(out=recip_sb, in_=sums_sb)
                            ps_bc = bcps.tile([128, QB], F32, tag="bc",
                                              name=f"bc{jq}_{g}_{jj}")
                            nc.tensor.matmul(
                                ps_bc[64 * rj:64 * (rj + 1), :], ones_row,
                                recip_sb, tile_position=(0, 64 * rj),
                                start=True, stop=True)
                            bc_sb = bcsb.tile([128, QB], F32, tag="bc",
                                              name=f"bs{jq}_{g}_{jj}")
                            nc.scalar.copy(
                                out=bc_sb[64 * rj:64 * (rj + 1), :],
                                in_=ps_bc[64 * rj:64 * (rj + 1), :])
                            nc.vector.tensor_mul(
                                z_sb[64 * rj:64 * (rj + 1), g,
                                     jq * QB:(jq + 1) * QB],
                                ps_z[64 * rj:64 * (rj + 1), :],
                                bc_sb[64 * rj:64 * (rj + 1), :])

                    for it in range(4 * jq, 4 * jq + 4):
                        for db in range(4):
                            ps_o = ops.tile([128, 512], F32, tag="o",
                                            name=f"o{it}_{db}")
                            for g in range(2):
                                nc.tensor.matmul(
                                    ps_o, z_sb[:, g, it * 128:(it + 1) * 128],
                                    wo_sb[:, g, db * 512:(db + 1) * 512],
                                    start=(g == 0), stop=(g == 1))
                            o_sb = outsb.tile([128, 512], F32, tag="o",
                                              name=f"os{it}_{db}")
                            nc.vector.tensor_copy(out=o_sb, in_=ps_o)
                            nc.sync.dma_start(
                                out=o_d[it * 128:(it + 1) * 128,
                                        db * 512:(db + 1) * 512],
                                in_=o_sb)

    nc.compile()
    return nc


def get_program():
    if "nc" not in _CACHE:
        _CACHE["nc"] = _build_program()
    return _CACHE["nc"]


def make_in_maps(normalized_resid_pre, W_Q, W_K, W_V, W_O):
    x = normalized_resid_pre
    x = np.ascontiguousarray(np.asarray(x, np.float32))
    W_Q = np.asarray(W_Q, np.float32)
    W_K = np.asarray(W_K, np.float32)
    W_V = np.asarray(W_V, np.float32)
    W_O = np.asarray(W_O, np.float32)
    wqe = W_Q.reshape(NKV, GQ, D, HD).sum(1) * (1.0 / np.sqrt(HD))

    kk = np.arange(128)[:, None, None]
    mm = np.arange(4)[None, :, None]
    qq = np.arange(QB)[None, None, :]
    import ml_dtypes
    mask = np.where(mm * 128 + kk <= qq, 1.0, 0.0).astype(ml_dtypes.bfloat16)
    mask = np.ascontiguousarray(mask)

    in_maps = []
    for c in range(8):
        b, half = divmod(c, 2)
        heads = [4 * half + m for m in range(HPC)]
        xT = np.ascontiguousarray(x[b].T)
        wq = np.ascontiguousarray(np.concatenate([wqe[n] for n in heads], 1))
        wk = np.ascontiguousarray(np.concatenate([W_K[n] for n in heads], 1))
        wv = np.ascontiguousarray(np.concatenate([W_V[n] for n in heads], 1))
        # z rows within pair g: [0:64] = head 2g+1, [64:128] = head 2g
        wo = np.ascontiguousarray(np.concatenate(
            [W_O[heads[1]], W_O[heads[0]], W_O[heads[3]], W_O[heads[2]]], 0))
        import ml_dtypes
        bf = ml_dtypes.bfloat16
        in_maps.append({"xT": xT.astype(bf), "wq": wq.astype(bf),
                        "wk": wk.astype(bf), "wv": wv.astype(bf),
                        "wo": wo.astype(bf), "mask": mask})
    return in_maps


def run(in_maps, **kw):
    from concourse.bass_utils import run_bass_kernel_spmd
    return run_bass_kernel_spmd(get_program(), in_maps,
                                core_ids=list(range(8)), **kw)


def kernel(normalized_resid_pre, W_Q, W_K, W_V, W_O):
    in_maps = make_in_maps(normalized_resid_pre, W_Q, W_K, W_V, W_O)
    res = run(in_maps)
    out = np.empty((4, S, D), np.float32)
    for b in range(4):
        out[b] = res.results[2 * b]["o"] + res.results[2 * b + 1]["o"]
    return out



# revision 6
# speedup vs baseline: 1.3180x; 1.3180x over previous
"""Grouped multi-query attention (group axis summed) on 8 trn2 NeuronCores.

Math: reference sums the g axis of the grouped Q heads inside the score
einsum, so the whole module collapses to standard 8-head attention with
W_Qeff[n] = sum_g W_Q[4n+g] (and the 1/sqrt(64) score scale folded in).

Sharding: core c -> (batch b = c//2, kv-head half = c%2). Each core runs
4 heads of one batch and produces a full [2048, 2048] bf16 partial of the
output projection; the host sums the two halves per batch in f32.

Per-core pipeline (out = lhsT.T @ rhs, bf16 in / f32 acc):
  xT [d, t] host-pretransposed, staged per 512-t block; Q_T/K_T [128, t]
  head-pair rows built by projection matmuls; V [t, 4, 65] with a ones
  column per head. Scores S_T[k, q] = K @ Q^T as in the baseline, exp on
  the scalar engine (free-dim trimmed above the diagonal), causal mask as
  a single [128,128] multiplicative bf16 block on the diagonal tile only.
  PV runs transposed: out zT[q, 65] = p.T @ [V | 1] per 128-q chunk, so
  the softmax denominator lands in column 64 for free, normalization is a
  native per-partition reciprocal+tensor_scalar, and PV costs 65 free
  cycles per accumulation step instead of 512. A DMA-engine transpose
  (dma_start_transpose) restores z[hd, q] for the output projection.
  Projections are software-pipelined with attention (weighted round-robin
  emission) so the scalar engine's exp stream overlaps PE's projection
  matmuls; output-projection blocks are delayed to backfill the tail.
"""

import numpy as np

S = 2048
D = 2048
HD = 64
NKV = 8
GQ = 4   # grouped q heads per kv head (summed)
HPC = 4  # heads per core
TB = 512
QB = 512
NTB = S // TB
NDC = D // 128
NQB = S // QB
NKT = S // 128

_CACHE = {}


def _build_program():
    import concourse.bass as bass
    import concourse.tile as tile
    from concourse import bacc, mybir

    F32 = mybir.dt.float32
    BF16 = mybir.dt.bfloat16
    AF = mybir.ActivationFunctionType

    nc = bacc.Bacc("TRN2", target_bir_lowering=False, debug=False,
                   enable_asserts=False, num_devices=8)

    xT_d = nc.dram_tensor("xT", [D, S], BF16, kind="ExternalInput").ap()
    wq_d = nc.dram_tensor("wq", [D, HPC * HD], BF16, kind="ExternalInput").ap()
    wk_d = nc.dram_tensor("wk", [D, HPC * HD], BF16, kind="ExternalInput").ap()
    wv_d = nc.dram_tensor("wv", [D, HPC * HD], BF16, kind="ExternalInput").ap()
    wo_d = nc.dram_tensor("wo", [HPC * HD, D], BF16, kind="ExternalInput").ap()
    mask_d = nc.dram_tensor("mask", [128, 128], BF16, kind="ExternalInput").ap()
    o_d = nc.dram_tensor("o", [S, D], BF16, kind="ExternalOutput").ap()

    with tile.TileContext(nc) as tc, \
            nc.allow_low_precision(reason="bf16 matmul operands by design"):
        with (
            tc.tile_pool(name="singles", bufs=1) as singles,
            tc.tile_pool(name="persist", bufs=1) as persist,
            tc.tile_pool(name="xpool", bufs=2) as xpool,
            tc.tile_pool(name="work", bufs=4) as work,
            tc.tile_pool(name="ztnp", bufs=2) as ztnp,
            tc.tile_pool(name="rcp", bufs=8) as rcp,
            tc.tile_pool(name="osb", bufs=3) as osb,
            tc.tile_pool(name="pp", bufs=2, space=bass.MemorySpace.PSUM) as pp,
            tc.tile_pool(name="sps", bufs=2, space=bass.MemorySpace.PSUM) as sps,
            tc.tile_pool(name="ops", bufs=2, space=bass.MemorySpace.PSUM) as ops,
            tc.tile_pool(name="zpsA", bufs=1, space=bass.MemorySpace.PSUM) as zpsA,
            tc.tile_pool(name="zpsB", bufs=1, space=bass.MemorySpace.PSUM) as zpsB,
        ):
            wq_sb = singles.tile([128, NDC, HPC * HD], BF16, tag="wq")
            wk_sb = singles.tile([128, NDC, HPC * HD], BF16, tag="wk")
            wv_sb = singles.tile([128, NDC, HPC * HD], BF16, tag="wv")
            wo_sb = singles.tile([128, 2, D], BF16, tag="wo")
            mask_sb = singles.tile([128, 128], BF16, tag="mask")

            nc.sync.dma_start(out=wq_sb, in_=wq_d.rearrange("(c p) n -> p c n", p=128))
            nc.sync.dma_start(out=wk_sb, in_=wk_d.rearrange("(c p) n -> p c n", p=128))
            nc.sync.dma_start(out=wv_sb, in_=wv_d.rearrange("(c p) n -> p c n", p=128))
            nc.sync.dma_start(out=wo_sb, in_=wo_d.rearrange("(g p) d -> p g d", p=128))
            nc.sync.dma_start(out=mask_sb, in_=mask_d)

            qT_sb = persist.tile([128, 2, S], BF16, tag="qT")
            kT_sb = persist.tile([128, 2, S], BF16, tag="kT")
            v_sb = persist.tile([128, NKT, HPC, HD + 1], BF16, tag="v")
            z_sb = persist.tile([128, 2, S], BF16, tag="z")
            nc.vector.memset(v_sb[:, :, :, HD:HD + 1], 1.0)

            xT_r = xT_d.rearrange("(c p) t -> p c t", p=128)

            # ---- stream builders: lists of (pe_cost_cycles, emit_fn) ----

            def proj_stream(tb):
                items = []
                xtb = xpool.tile([128, NDC, TB], BF16, tag="xtb",
                                 name=f"xtb{tb}")

                def dma_chunk(c):
                    def f():
                        nc.sync.dma_start(
                            out=xtb[:, 4 * c:4 * (c + 1), :],
                            in_=xT_r[:, 4 * c:4 * (c + 1),
                                     tb * TB:(tb + 1) * TB])
                    return f
                for c in range(4):
                    items.append((0, dma_chunk(c)))

                for kind, g in [(0, 0), (1, 0), (0, 1), (1, 1)]:
                    w_sb = wq_sb if kind == 0 else wk_sb
                    dst = qT_sb if kind == 0 else kT_sb
                    psq = pp.tile([128, TB], F32, tag="pp",
                                  name=f"pp{tb}_{kind}_{g}")

                    def quarter(psq=psq, w_sb=w_sb, g=g, q4=0):
                        def f():
                            for dc in range(4 * q4, 4 * q4 + 4):
                                nc.tensor.matmul(
                                    psq, w_sb[:, dc, 128 * g:128 * (g + 1)],
                                    xtb[:, dc, :],
                                    start=(dc == 0), stop=(dc == NDC - 1),
                                    skip_group_check=True)
                        return f
                    for q4 in range(4):
                        items.append((2048, quarter(psq, w_sb, g, q4)))

                    def drain(psq=psq, dst=dst, g=g):
                        def f():
                            nc.scalar.copy(
                                out=dst[:, g, tb * TB:(tb + 1) * TB], in_=psq)
                        return f
                    items.append((0, drain(psq, dst, g)))

                for tt in range(4):
                    psv = pp.tile([128, HPC * HD], F32, tag="pp",
                                  name=f"ppv{tb}_{tt}")

                    def vquarter(psv=psv, tt=tt, q4=0):
                        def f():
                            for dc in range(4 * q4, 4 * q4 + 4):
                                nc.tensor.matmul(
                                    psv, xtb[:, dc, tt * 128:(tt + 1) * 128],
                                    wv_sb[:, dc, :],
                                    start=(dc == 0), stop=(dc == NDC - 1),
                                    skip_group_check=True)
                        return f
                    for q4 in range(4):
                        items.append((1024, vquarter(psv, tt, q4)))

                    def vdrain(psv=psv, tt=tt):
                        def f():
                            nc.vector.tensor_copy(
                                out=v_sb[:, 4 * tb + tt, :, 0:HD],
                                in_=psv.rearrange("p (h e) -> p h e", h=HPC))
                        return f
                    items.append((0, vdrain(psv, tt)))
                return items

            def att_stream(jq):
                items = []
                nkt = 4 * (jq + 1)
                zt = {}
                ztn = {}

                def mk_ik(g, ik):
                    def f():
                        if ik == 0:
                            zt[(g, 0)] = zpsA.tile([128, 4, HD + 1], F32,
                                                   tag="zA", name=f"zA{jq}_{g}")
                            zt[(g, 1)] = zpsB.tile([128, 4, HD + 1], F32,
                                                   tag="zB", name=f"zB{jq}_{g}")
                            ztn[g] = ztnp.tile([128, 4, 128], BF16, tag="ztn",
                                               name=f"ztn{jq}_{g}")
                        m = ik - 4 * jq  # diagonal block index (>=0 on diag)
                        off = 128 * m if m > 0 else 0
                        p2 = []
                        for jj in range(2):
                            ps_s = sps.tile([128, QB], F32, tag="s",
                                            name=f"s{jq}_{g}_{ik}_{jj}")
                            nc.tensor.matmul(
                                ps_s[:, off:],
                                kT_sb[64 * jj:64 * (jj + 1), g,
                                      ik * 128:(ik + 1) * 128],
                                qT_sb[64 * jj:64 * (jj + 1), g,
                                      jq * QB + off:(jq + 1) * QB],
                                start=True, stop=True)
                            p_sb = work.tile([128, QB], BF16, tag="p",
                                             name=f"p{jq}_{g}_{ik}_{jj}")
                            nc.scalar.activation(out=p_sb[:, off:],
                                                 in_=ps_s[:, off:], func=AF.Exp)
                            if m >= 0:
                                nc.gpsimd.tensor_mul(
                                    p_sb[:, off:off + 128],
                                    p_sb[:, off:off + 128], mask_sb)
                            p2.append(p_sb)
                        for jj, p_sb in enumerate(p2):
                            h = 2 * g + jj
                            for qc in range(4):
                                if ik > 4 * jq + qc:
                                    continue
                                # one accumulation group per bank: start marks
                                # the whole 2KB zero region, so only the first
                                # write may carry start and only the last stop
                                nc.tensor.matmul(
                                    zt[(g, jj)][:, qc, :],
                                    p_sb[:, qc * 128:(qc + 1) * 128],
                                    v_sb[:, ik, h, :],
                                    start=(ik == 0 and qc == 0),
                                    stop=(ik == 4 * jq + 3 and qc == 3),
                                    skip_group_check=True)
                        if m >= 0:
                            qc = m
                            for jj in range(2):
                                rc = rcp.tile([128, 1], F32, tag="rc",
                                              name=f"rc{jq}_{g}_{qc}_{jj}")
                                nc.vector.reciprocal(
                                    out=rc, in_=zt[(g, jj)][:, qc, HD:HD + 1])
                                nc.vector.tensor_scalar_mul(
                                    ztn[g][:, qc, 64 * jj:64 * (jj + 1)],
                                    zt[(g, jj)][:, qc, 0:HD], rc)
                        if ik == nkt - 1:
                            nc.sync.dma_start_transpose(
                                out=z_sb[:, g, jq * QB:(jq + 1) * QB]
                                .rearrange("p (a b) -> p a b", a=4),
                                in_=ztn[g].rearrange("p a b -> p (a b)"))
                    m = ik - 4 * jq
                    off = 128 * m if m > 0 else 0
                    nvalid = 4 - max(0, m)
                    cost = 2 * (QB - off) + 2 * nvalid * (HD + 1)
                    return (cost, f)

                for g in range(2):
                    for ik in range(nkt):
                        items.append(mk_ik(g, ik))
                return items

            def p3_stream(jq):
                items = []

                def mk(it, db):
                    def f():
                        ps_o = ops.tile([128, 512], F32, tag="o",
                                        name=f"o{it}_{db}")
                        for g in range(2):
                            nc.tensor.matmul(
                                ps_o, z_sb[:, g, it * 128:(it + 1) * 128],
                                wo_sb[:, g, db * 512:(db + 1) * 512],
                                start=(g == 0), stop=(g == 1),
                                skip_group_check=True)
                        o_sb = osb.tile([128, 512], BF16, tag="o",
                                        name=f"os{it}_{db}")
                        nc.vector.tensor_copy(out=o_sb, in_=ps_o)
                        nc.sync.dma_start(
                            out=o_d[it * 128:(it + 1) * 128,
                                    db * 512:(db + 1) * 512],
                            in_=o_sb)
                    return (1024, f)

                for it in range(4 * jq, 4 * jq + 4):
                    for db in range(4):
                        items.append(mk(it, db))
                return items

            def merge(*streams):
                streams = [list(s) for s in streams if s]
                totals = [max(1, sum(c for c, _ in s)) for s in streams]
                done = [0.0] * len(streams)
                idx = [0] * len(streams)
                while True:
                    best, bestf = None, None
                    for i, s in enumerate(streams):
                        if idx[i] < len(s):
                            frac = done[i] / totals[i]
                            if best is None or frac < bestf:
                                best, bestf = i, frac
                    if best is None:
                        break
                    c, f = streams[best][idx[best]]
                    f()
                    done[best] += c
                    idx[best] += 1

            merge(proj_stream(0))
            merge(att_stream(0), proj_stream(1))
            merge(att_stream(1), proj_stream(2))
            merge(att_stream(2), proj_stream(3), p3_stream(0))
            # p3(3) consumes att(3)'s transposes — must be emitted after them
            merge(att_stream(3), p3_stream(1), p3_stream(2))
            merge(p3_stream(3))

    nc.compile()
    return nc


def get_program():
    if "nc" not in _CACHE:
        _CACHE["nc"] = _build_program()
    return _CACHE["nc"]


def make_in_maps(normalized_resid_pre, W_Q, W_K, W_V, W_O):
    import ml_dtypes
    bf = ml_dtypes.bfloat16

    x = np.ascontiguousarray(np.asarray(normalized_resid_pre, np.float32))
    W_Q = np.asarray(W_Q, np.float32)
    W_K = np.asarray(W_K, np.float32)
    W_V = np.asarray(W_V, np.float32)
    W_O = np.asarray(W_O, np.float32)
    wqe = W_Q.reshape(NKV, GQ, D, HD).sum(1) * (1.0 / np.sqrt(HD))

    kk = np.arange(128)[:, None]
    qq = np.arange(128)[None, :]
    mask = np.ascontiguousarray(np.where(kk <= qq, 1.0, 0.0).astype(bf))

    in_maps = []
    for c in range(8):
        b, half = divmod(c, 2)
        heads = [4 * half + m for m in range(HPC)]
        xT = np.ascontiguousarray(x[b].T)
        wq = np.ascontiguousarray(np.concatenate([wqe[n] for n in heads], 1))
        wk = np.ascontiguousarray(np.concatenate([W_K[n] for n in heads], 1))
        wv = np.ascontiguousarray(np.concatenate([W_V[n] for n in heads], 1))
        wo = np.ascontiguousarray(np.concatenate([W_O[n] for n in heads], 0))
        in_maps.append({"xT": xT.astype(bf), "wq": wq.astype(bf),
                        "wk": wk.astype(bf), "wv": wv.astype(bf),
                        "wo": wo.astype(bf), "mask": mask})
    return in_maps


def run(in_maps, **kw):
    from concourse.bass_utils import run_bass_kernel_spmd
    return run_bass_kernel_spmd(get_program(), in_maps,
                                core_ids=list(range(8)), **kw)


def kernel(normalized_resid_pre, W_Q, W_K, W_V, W_O):
    in_maps = make_in_maps(normalized_resid_pre, W_Q, W_K, W_V, W_O)
    res = run(in_maps)
    out = np.empty((4, S, D), np.float32)
    for b in range(4):
        out[b] = (res.results[2 * b]["o"].astype(np.float32)
                  + res.results[2 * b + 1]["o"].astype(np.float32))
    return out


# revision 10
# speedup vs baseline: 1.3428x; 1.0188x over previous
"""Grouped multi-query attention (group axis summed) on 8 trn2 NeuronCores.

Math: reference sums the g axis of the grouped Q heads inside the score
einsum, so the whole module collapses to standard 8-head attention with
W_Qeff[n] = sum_g W_Q[4n+g] (and the 1/sqrt(64) score scale folded in).

Sharding: core c -> (batch b = c//2, kv-head half = c%2). Each core runs
4 heads of one batch and produces a full [2048, 2048] bf16 partial of the
output projection; the host sums the two halves per batch in f32.

Per-core pipeline (out = lhsT.T @ rhs, bf16 in / f32 acc):
  xT [d, t] host-pretransposed, staged per 512-t block; Q_T/K_T [128, t]
  head-pair rows built by projection matmuls; V [t, 4, 65] with a ones
  column per head. Scores S_T[k, q] = K @ Q^T as in the baseline, exp on
  the scalar engine (free-dim trimmed above the diagonal), causal mask as
  a single [128,128] multiplicative bf16 block on the diagonal tile only.
  PV runs transposed: out zT[q, 65] = p.T @ [V | 1] per 128-q chunk, so
  the softmax denominator lands in column 64 for free, normalization is a
  native per-partition reciprocal+tensor_scalar, and PV costs 65 free
  cycles per accumulation step instead of 512. A DMA-engine transpose
  (dma_start_transpose) restores z[hd, q] for the output projection.
  Projections are software-pipelined with attention (weighted round-robin
  emission) so the scalar engine's exp stream overlaps PE's projection
  matmuls; output-projection blocks are delayed to backfill the tail.
"""

import numpy as np

S = 2048
D = 2048
HD = 64
NKV = 8
GQ = 4   # grouped q heads per kv head (summed)
HPC = 4  # heads per core
TB = 512
QB = 512
NTB = S // TB
NDC = D // 128
NQB = S // QB
NKT = S // 128

_CACHE = {}


def _build_program():
    import concourse.bass as bass
    import concourse.tile as tile
    from concourse import bacc, mybir

    F32 = mybir.dt.float32
    BF16 = mybir.dt.bfloat16
    AF = mybir.ActivationFunctionType

    nc = bacc.Bacc("TRN2", target_bir_lowering=False, debug=False,
                   enable_asserts=False, num_devices=8)

    xT_d = nc.dram_tensor("xT", [D, S], BF16, kind="ExternalInput").ap()
    wq_d = nc.dram_tensor("wq", [D, HPC * HD], BF16, kind="ExternalInput").ap()
    wk_d = nc.dram_tensor("wk", [D, HPC * HD], BF16, kind="ExternalInput").ap()
    wv_d = nc.dram_tensor("wv", [D, HPC * HD], BF16, kind="ExternalInput").ap()
    wo_d = nc.dram_tensor("wo", [HPC * HD, D], BF16, kind="ExternalInput").ap()
    mask_d = nc.dram_tensor("mask", [128, 128], BF16, kind="ExternalInput").ap()
    o_d = nc.dram_tensor("o", [S, D], BF16, kind="ExternalOutput").ap()

    with tile.TileContext(nc) as tc, \
            nc.allow_low_precision(reason="bf16 matmul operands by design"):
        with (
            tc.tile_pool(name="singles", bufs=1) as singles,
            tc.tile_pool(name="persist", bufs=1) as persist,
            tc.tile_pool(name="xpool", bufs=2) as xpool,
            tc.tile_pool(name="work", bufs=4) as work,
            tc.tile_pool(name="ztnp", bufs=2) as ztnp,
            tc.tile_pool(name="rcp", bufs=8) as rcp,
            tc.tile_pool(name="osb", bufs=3) as osb,
            tc.tile_pool(name="pp", bufs=2, space=bass.MemorySpace.PSUM) as pp,
            tc.tile_pool(name="sps", bufs=4, space=bass.MemorySpace.PSUM) as sps,
            tc.tile_pool(name="zpsA", bufs=1, space=bass.MemorySpace.PSUM) as zpsA,
            tc.tile_pool(name="zpsB", bufs=1, space=bass.MemorySpace.PSUM) as zpsB,
        ):
            wq_sb = singles.tile([128, NDC, HPC * HD], BF16, tag="wq")
            wk_sb = singles.tile([128, NDC, HPC * HD], BF16, tag="wk")
            wv_sb = singles.tile([128, NDC, HPC * HD], BF16, tag="wv")
            wo_sb = singles.tile([128, 2, D], BF16, tag="wo")
            mask_sb = singles.tile([128, 128], BF16, tag="mask")

            qT_sb = persist.tile([128, 2, S], BF16, tag="qT")
            kT_sb = persist.tile([128, 2, S], BF16, tag="kT")
            v_sb = persist.tile([128, NKT, HPC, HD + 1], BF16, tag="v")
            z_sb = persist.tile([128, 2, S], BF16, tag="z")
            nc.vector.memset(v_sb[:, :, :, HD:HD + 1], 1.0)

            xT_r = xT_d.rearrange("(c p) t -> p c t", p=128)

            # ---- stream builders: lists of (pe_cost_cycles, emit_fn) ----

            def proj_stream(tb):
                items = []
                xtb = xpool.tile([128, NDC, TB], BF16, tag="xtb",
                                 name=f"xtb{tb}")

                def dma_chunk(c):
                    def f():
                        nc.sync.dma_start(
                            out=xtb[:, 4 * c:4 * (c + 1), :],
                            in_=xT_r[:, 4 * c:4 * (c + 1),
                                     tb * TB:(tb + 1) * TB])
                    return f
                for c in range(4):
                    items.append((0, dma_chunk(c)))

                for kind, g in [(0, 0), (1, 0), (0, 1), (1, 1)]:
                    w_sb = wq_sb if kind == 0 else wk_sb
                    dst = qT_sb if kind == 0 else kT_sb
                    psq = pp.tile([128, TB], F32, tag="pp",
                                  name=f"pp{tb}_{kind}_{g}")

                    def quarter(psq=psq, w_sb=w_sb, g=g, q4=0):
                        def f():
                            for dc in range(4 * q4, 4 * q4 + 4):
                                nc.tensor.matmul(
                                    psq, w_sb[:, dc, 128 * g:128 * (g + 1)],
                                    xtb[:, dc, :],
                                    start=(dc == 0), stop=(dc == NDC - 1),
                                    skip_group_check=True)
                        return f
                    for q4 in range(4):
                        items.append((2048, quarter(psq, w_sb, g, q4)))

                    def drain(psq=psq, dst=dst, g=g):
                        def f():
                            nc.scalar.copy(
                                out=dst[:, g, tb * TB:(tb + 1) * TB], in_=psq)
                        return f
                    items.append((0, drain(psq, dst, g)))

                for tt in range(4):
                    psv = pp.tile([128, HPC * HD], F32, tag="pp",
                                  name=f"ppv{tb}_{tt}")

                    def vquarter(psv=psv, tt=tt, q4=0):
                        def f():
                            for dc in range(4 * q4, 4 * q4 + 4):
                                nc.tensor.matmul(
                                    psv, xtb[:, dc, tt * 128:(tt + 1) * 128],
                                    wv_sb[:, dc, :],
                                    start=(dc == 0), stop=(dc == NDC - 1),
                                    skip_group_check=True)
                        return f
                    for q4 in range(4):
                        items.append((1024, vquarter(psv, tt, q4)))

                    def vdrain(psv=psv, tt=tt):
                        def f():
                            nc.vector.tensor_copy(
                                out=v_sb[:, 4 * tb + tt, :, 0:HD],
                                in_=psv.rearrange("p (h e) -> p h e", h=HPC))
                        return f
                    items.append((0, vdrain(psv, tt)))
                return items

            def att_stream(jq):
                items = []
                nkt = 4 * (jq + 1)
                zt = {}
                ztn = {}

                def mk_ik(g, ik):
                    def f():
                        if ik == 0:
                            zt[(g, 0)] = zpsA.tile([128, 4, HD + 1], F32,
                                                   tag="zA", name=f"zA{jq}_{g}")
                            zt[(g, 1)] = zpsB.tile([128, 4, HD + 1], F32,
                                                   tag="zB", name=f"zB{jq}_{g}")
                            ztn[g] = ztnp.tile([128, 4, 128], BF16, tag="ztn",
                                               name=f"ztn{jq}_{g}")
                        m = ik - 4 * jq  # diagonal block index (>=0 on diag)
                        off = 128 * m if m > 0 else 0
                        p2 = []
                        for jj in range(2):
                            ps_s = sps.tile([128, QB], F32, tag="s",
                                            name=f"s{jq}_{g}_{ik}_{jj}")
                            nc.tensor.matmul(
                                ps_s[:, off:],
                                kT_sb[64 * jj:64 * (jj + 1), g,
                                      ik * 128:(ik + 1) * 128],
                                qT_sb[64 * jj:64 * (jj + 1), g,
                                      jq * QB + off:(jq + 1) * QB],
                                start=True, stop=True)
                            p_sb = work.tile([128, QB], BF16, tag="p",
                                             name=f"p{jq}_{g}_{ik}_{jj}")
                            nc.scalar.activation(out=p_sb[:, off:],
                                                 in_=ps_s[:, off:], func=AF.Exp)
                            if m >= 0:
                                nc.gpsimd.tensor_mul(
                                    p_sb[:, off:off + 128],
                                    p_sb[:, off:off + 128], mask_sb)
                            p2.append(p_sb)
                        for jj, p_sb in enumerate(p2):
                            h = 2 * g + jj
                            for qc in range(4):
                                if ik > 4 * jq + qc:
                                    continue
                                # one accumulation group per bank: start marks
                                # the whole 2KB zero region, so only the first
                                # write may carry start and only the last stop
                                nc.tensor.matmul(
                                    zt[(g, jj)][:, qc, :],
                                    p_sb[:, qc * 128:(qc + 1) * 128],
                                    v_sb[:, ik, h, :],
                                    start=(ik == 0 and qc == 0),
                                    stop=(ik == 4 * jq + 3 and qc == 3),
                                    skip_group_check=True)
                        if m >= 0:
                            qc = m
                            for jj in range(2):
                                rc = rcp.tile([128, 1], F32, tag="rc",
                                              name=f"rc{jq}_{g}_{qc}_{jj}")
                                nc.vector.reciprocal(
                                    out=rc, in_=zt[(g, jj)][:, qc, HD:HD + 1])
                                nc.vector.tensor_scalar_mul(
                                    ztn[g][:, qc, 64 * jj:64 * (jj + 1)],
                                    zt[(g, jj)][:, qc, 0:HD], rc)
                        if ik == nkt - 1:
                            nc.sync.dma_start_transpose(
                                out=z_sb[:, g, jq * QB:(jq + 1) * QB]
                                .rearrange("p (a b) -> p a b", a=4),
                                in_=ztn[g].rearrange("p a b -> p (a b)"))
                    m = ik - 4 * jq
                    off = 128 * m if m > 0 else 0
                    nvalid = 4 - max(0, m)
                    cost = 2 * (QB - off) + 2 * nvalid * (HD + 1)
                    return (cost, f)

                for g in range(2):
                    for ik in range(nkt):
                        items.append(mk_ik(g, ik))
                return items

            def p3_stream(jq):
                # output projection runs in the last window, after all
                # projections: ps_o reuses the freed pp ring (and, for the
                # post-attention jq==3 tail, also the sps ring); drains
                # alternate DVE/Act so neither becomes the cadence limit
                items = []

                def mk(it, db, i):
                    def f():
                        if jq == 3 and i % 2 == 1:
                            ps_o = sps.tile([128, 512], F32, tag="s",
                                            name=f"o{it}_{db}")
                        else:
                            ps_o = pp.tile([128, 512], F32, tag="pp",
                                           name=f"o{it}_{db}")
                        for g in range(2):
                            nc.tensor.matmul(
                                ps_o, z_sb[:, g, it * 128:(it + 1) * 128],
                                wo_sb[:, g, db * 512:(db + 1) * 512],
                                start=(g == 0), stop=(g == 1),
                                skip_group_check=True)
                        o_sb = osb.tile([128, 512], BF16, tag="o",
                                        name=f"os{it}_{db}")
                        if i % 2 == 0:
                            nc.vector.tensor_copy(out=o_sb, in_=ps_o)
                        else:
                            nc.scalar.copy(out=o_sb, in_=ps_o)
                        nc.sync.dma_start(
                            out=o_d[it * 128:(it + 1) * 128,
                                    db * 512:(db + 1) * 512],
                            in_=o_sb)
                    return (1024, f)

                i = 0
                for it in range(4 * jq, 4 * jq + 4):
                    for db in range(4):
                        items.append(mk(it, db, i))
                        i += 1
                return items

            def merge(*streams):
                streams = [list(s) for s in streams if s]
                totals = [max(1, sum(c for c, _ in s)) for s in streams]
                done = [0.0] * len(streams)
                idx = [0] * len(streams)
                while True:
                    best, bestf = None, None
                    for i, s in enumerate(streams):
                        if idx[i] < len(s):
                            frac = done[i] / totals[i]
                            if best is None or frac < bestf:
                                best, bestf = i, frac
                    if best is None:
                        break
                    c, f = streams[best][idx[best]]
                    f()
                    done[best] += c
                    idx[best] += 1

            # startup: first x chunk and q/k weights first so PE starts
            # ~6us in; wv/mask/wo are not needed until later
            s0 = proj_stream(0)
            s0[0][1]()  # xtb(0) chunk 0 DMA
            nc.sync.dma_start(out=wq_sb, in_=wq_d.rearrange("(c p) n -> p c n", p=128))
            nc.sync.dma_start(out=wk_sb, in_=wk_d.rearrange("(c p) n -> p c n", p=128))
            for c in range(1, 4):
                s0[c][1]()  # xtb(0) chunks 1-3
            nc.sync.dma_start(out=wv_sb, in_=wv_d.rearrange("(c p) n -> p c n", p=128))
            nc.sync.dma_start(out=mask_sb, in_=mask_d)
            nc.sync.dma_start(out=wo_sb, in_=wo_d.rearrange("(g p) d -> p g d", p=128))
            merge(s0[4:])
            merge(att_stream(0), proj_stream(1))
            merge(att_stream(1), proj_stream(2))
            merge(att_stream(2), proj_stream(3))
            # all output projection backfills the Act-bound att(3) window;
            # p3(3) consumes att(3)'s transposes — emitted strictly after
            merge(att_stream(3), p3_stream(0), p3_stream(1), p3_stream(2))
            merge(p3_stream(3))

    nc.compile()
    return nc


def get_program():
    if "nc" not in _CACHE:
        _CACHE["nc"] = _build_program()
    return _CACHE["nc"]


def make_in_maps(normalized_resid_pre, W_Q, W_K, W_V, W_O):
    import ml_dtypes
    bf = ml_dtypes.bfloat16

    x = np.ascontiguousarray(np.asarray(normalized_resid_pre, np.float32))
    W_Q = np.asarray(W_Q, np.float32)
    W_K = np.asarray(W_K, np.float32)
    W_V = np.asarray(W_V, np.float32)
    W_O = np.asarray(W_O, np.float32)
    wqe = W_Q.reshape(NKV, GQ, D, HD).sum(1) * (1.0 / np.sqrt(HD))

    kk = np.arange(128)[:, None]
    qq = np.arange(128)[None, :]
    mask = np.ascontiguousarray(np.where(kk <= qq, 1.0, 0.0).astype(bf))

    in_maps = []
    for c in range(8):
        b, half = divmod(c, 2)
        heads = [4 * half + m for m in range(HPC)]
        xT = np.ascontiguousarray(x[b].T)
        wq = np.ascontiguousarray(np.concatenate([wqe[n] for n in heads], 1))
        wk = np.ascontiguousarray(np.concatenate([W_K[n] for n in heads], 1))
        wv = np.ascontiguousarray(np.concatenate([W_V[n] for n in heads], 1))
        wo = np.ascontiguousarray(np.concatenate([W_O[n] for n in heads], 0))
        in_maps.append({"xT": xT.astype(bf), "wq": wq.astype(bf),
                        "wk": wk.astype(bf), "wv": wv.astype(bf),
                        "wo": wo.astype(bf), "mask": mask})
    return in_maps


def run(in_maps, **kw):
    from concourse.bass_utils import run_bass_kernel_spmd
    return run_bass_kernel_spmd(get_program(), in_maps,
                                core_ids=list(range(8)), **kw)


def kernel(normalized_resid_pre, W_Q, W_K, W_V, W_O):
    in_maps = make_in_maps(normalized_resid_pre, W_Q, W_K, W_V, W_O)
    res = run(in_maps)
    out = np.empty((4, S, D), np.float32)
    for b in range(4):
        out[b] = (res.results[2 * b]["o"].astype(np.float32)
                  + res.results[2 * b + 1]["o"].astype(np.float32))
    return out


# revision 13
# speedup vs baseline: 1.4639x; 1.0902x over previous
"""Grouped multi-query attention (group axis summed) on 8 trn2 NeuronCores.

Math: reference sums the g axis of the grouped Q heads inside the score
einsum, so the whole module collapses to standard 8-head attention with
W_Qeff[n] = sum_g W_Q[4n+g] (and the 1/sqrt(64) score scale folded in).

Sharding: core c -> (batch b = c//2, kv-head half = c%2). Each core runs
4 heads of one batch and produces a full [2048, 2048] bf16 partial of the
output projection; the host sums the two halves per batch in f32.

Per-core pipeline (out = lhsT.T @ rhs, bf16 in / f32 acc):
  xT [d, t] host-pretransposed, staged per 512-t block; Q_T/K_T [128, t]
  head-pair rows built by projection matmuls; V [t, 4, 65] with a ones
  column per head. Scores S_T[k, q] = K @ Q^T as in the baseline, exp on
  the scalar engine (free-dim trimmed above the diagonal), causal mask as
  a single [128,128] multiplicative bf16 block on the diagonal tile only.
  PV runs transposed: out zT[q, 65] = p.T @ [V | 1] per 128-q chunk, so
  the softmax denominator lands in column 64 for free, normalization is a
  native per-partition reciprocal+tensor_scalar, and PV costs 65 free
  cycles per accumulation step instead of 512. A DMA-engine transpose
  (dma_start_transpose) restores z[hd, q] for the output projection.
  Projections are software-pipelined with attention (weighted round-robin
  emission) so the scalar engine's exp stream overlaps PE's projection
  matmuls; output-projection blocks are delayed to backfill the tail.
"""

import numpy as np

S = 2048
D = 2048
HD = 64
NKV = 8
GQ = 4   # grouped q heads per kv head (summed)
HPC = 4  # heads per core
TB = 512
QB = 512
NTB = S // TB
NDC = D // 128
NQB = S // QB
NKT = S // 128

_CACHE = {}


def _build_program():
    import concourse.bass as bass
    import concourse.tile as tile
    from concourse import bacc, mybir

    F32 = mybir.dt.float32
    BF16 = mybir.dt.bfloat16
    AF = mybir.ActivationFunctionType

    nc = bacc.Bacc("TRN2", target_bir_lowering=False, debug=False,
                   enable_asserts=False, num_devices=8)

    xT_d = nc.dram_tensor("xT", [D, S], BF16, kind="ExternalInput").ap()
    wq_d = nc.dram_tensor("wq", [D, HPC * HD], BF16, kind="ExternalInput").ap()
    wk_d = nc.dram_tensor("wk", [D, HPC * HD], BF16, kind="ExternalInput").ap()
    wv_d = nc.dram_tensor("wv", [D, HPC * HD], BF16, kind="ExternalInput").ap()
    wo_d = nc.dram_tensor("wo", [HPC * HD, D], BF16, kind="ExternalInput").ap()
    mask_d = nc.dram_tensor("mask", [128, 128], BF16, kind="ExternalInput").ap()
    o_d = nc.dram_tensor("o", [S, D], BF16, kind="ExternalOutput").ap()

    with tile.TileContext(nc) as tc, \
            nc.allow_low_precision(reason="bf16 matmul operands by design"):
        with (
            tc.tile_pool(name="singles", bufs=1) as singles,
            tc.tile_pool(name="persist", bufs=1) as persist,
            tc.tile_pool(name="xpool", bufs=2) as xpool,
            tc.tile_pool(name="work", bufs=4) as work,
            tc.tile_pool(name="ztnp", bufs=2) as ztnp,
            tc.tile_pool(name="rcp", bufs=8) as rcp,
            tc.tile_pool(name="osb", bufs=3) as osb,
            tc.tile_pool(name="pp", bufs=2, space=bass.MemorySpace.PSUM) as pp,
            tc.tile_pool(name="sps", bufs=4, space=bass.MemorySpace.PSUM) as sps,
            tc.tile_pool(name="zpsA", bufs=1, space=bass.MemorySpace.PSUM) as zpsA,
            tc.tile_pool(name="zpsB", bufs=1, space=bass.MemorySpace.PSUM) as zpsB,
        ):
            wq_sb = singles.tile([128, NDC, HPC * HD], BF16, tag="wq")
            wk_sb = singles.tile([128, NDC, HPC * HD], BF16, tag="wk")
            wv_sb = singles.tile([128, NDC, HPC * HD], BF16, tag="wv")
            wo_sb = singles.tile([128, 2, D], BF16, tag="wo")
            mask_sb = singles.tile([128, 128], BF16, tag="mask")

            qT_sb = persist.tile([128, 2, S], BF16, tag="qT")
            kT_sb = persist.tile([128, 2, S], BF16, tag="kT")
            v_sb = persist.tile([128, NKT, HPC, HD + 1], BF16, tag="v")
            z_sb = persist.tile([128, 2, S], BF16, tag="z")
            nc.vector.memset(v_sb[:, :, :, HD:HD + 1], 1.0)

            xT_r = xT_d.rearrange("(c p) t -> p c t", p=128)

            # ---- stream builders: lists of (pe_cost_cycles, emit_fn) ----

            def proj_stream(tb):
                items = []
                xtb = xpool.tile([128, NDC, TB], BF16, tag="xtb",
                                 name=f"xtb{tb}")

                def dma_chunk(c):
                    def f():
                        nc.sync.dma_start(
                            out=xtb[:, 4 * c:4 * (c + 1), :],
                            in_=xT_r[:, 4 * c:4 * (c + 1),
                                     tb * TB:(tb + 1) * TB])
                    return f
                for c in range(4):
                    items.append((0, dma_chunk(c)))

                for kind, g in [(0, 0), (1, 0), (0, 1), (1, 1)]:
                    w_sb = wq_sb if kind == 0 else wk_sb
                    dst = qT_sb if kind == 0 else kT_sb
                    psq = pp.tile([128, TB], F32, tag="pp",
                                  name=f"pp{tb}_{kind}_{g}")

                    def quarter(psq=psq, w_sb=w_sb, g=g, q4=0):
                        def f():
                            for dc in range(4 * q4, 4 * q4 + 4):
                                nc.tensor.matmul(
                                    psq, w_sb[:, dc, 128 * g:128 * (g + 1)],
                                    xtb[:, dc, :],
                                    start=(dc == 0), stop=(dc == NDC - 1),
                                    skip_group_check=True)
                        return f
                    for q4 in range(4):
                        items.append((2048, quarter(psq, w_sb, g, q4)))

                    def drain(psq=psq, dst=dst, g=g):
                        def f():
                            nc.vector.tensor_copy(
                                out=dst[:, g, tb * TB:(tb + 1) * TB], in_=psq)
                        return f
                    items.append((0, drain(psq, dst, g)))

                for tt in range(4):
                    psv = pp.tile([128, HPC * HD], F32, tag="pp",
                                  name=f"ppv{tb}_{tt}")

                    def vquarter(psv=psv, tt=tt, q4=0):
                        def f():
                            for dc in range(4 * q4, 4 * q4 + 4):
                                nc.tensor.matmul(
                                    psv, xtb[:, dc, tt * 128:(tt + 1) * 128],
                                    wv_sb[:, dc, :],
                                    start=(dc == 0), stop=(dc == NDC - 1),
                                    skip_group_check=True)
                        return f
                    for q4 in range(4):
                        items.append((1024, vquarter(psv, tt, q4)))

                    def vdrain(psv=psv, tt=tt):
                        def f():
                            nc.vector.tensor_copy(
                                out=v_sb[:, 4 * tb + tt, :, 0:HD],
                                in_=psv.rearrange("p (h e) -> p h e", h=HPC))
                        return f
                    items.append((0, vdrain(psv, tt)))
                return items

            def att_stream(jq):
                items = []
                nkt = 4 * (jq + 1)
                zt = {}
                ztn = {}

                def mk_ik(g, ik):
                    def f():
                        if ik == 0:
                            zt[(g, 0)] = zpsA.tile([128, 4, HD + 1], F32,
                                                   tag="zA", name=f"zA{jq}_{g}")
                            zt[(g, 1)] = zpsB.tile([128, 4, HD + 1], F32,
                                                   tag="zB", name=f"zB{jq}_{g}")
                            ztn[g] = ztnp.tile([128, 4, 128], BF16, tag="ztn",
                                               name=f"ztn{jq}_{g}")
                        m = ik - 4 * jq  # diagonal block index (>=0 on diag)
                        off = 128 * m if m > 0 else 0
                        p2 = []
                        for jj in range(2):
                            ps_s = sps.tile([128, QB], F32, tag="s",
                                            name=f"s{jq}_{g}_{ik}_{jj}")
                            nc.tensor.matmul(
                                ps_s[:, off:],
                                kT_sb[64 * jj:64 * (jj + 1), g,
                                      ik * 128:(ik + 1) * 128],
                                qT_sb[64 * jj:64 * (jj + 1), g,
                                      jq * QB + off:(jq + 1) * QB],
                                start=True, stop=True)
                            p_sb = work.tile([128, QB], BF16, tag="p",
                                             name=f"p{jq}_{g}_{ik}_{jj}")
                            nc.scalar.activation(out=p_sb[:, off:],
                                                 in_=ps_s[:, off:], func=AF.Exp)
                            if m >= 0:
                                nc.gpsimd.tensor_mul(
                                    p_sb[:, off:off + 128],
                                    p_sb[:, off:off + 128], mask_sb)
                            p2.append(p_sb)
                        for jj, p_sb in enumerate(p2):
                            h = 2 * g + jj
                            for qc in range(4):
                                if ik > 4 * jq + qc:
                                    continue
                                # one accumulation group per bank: start marks
                                # the whole 2KB zero region, so only the first
                                # write may carry start and only the last stop
                                nc.tensor.matmul(
                                    zt[(g, jj)][:, qc, :],
                                    p_sb[:, qc * 128:(qc + 1) * 128],
                                    v_sb[:, ik, h, :],
                                    start=(ik == 0 and qc == 0),
                                    stop=(ik == 4 * jq + 3 and qc == 3),
                                    skip_group_check=True)
                        if m >= 0:
                            qc = m
                            for jj in range(2):
                                rc = rcp.tile([128, 1], F32, tag="rc",
                                              name=f"rc{jq}_{g}_{qc}_{jj}")
                                nc.vector.reciprocal(
                                    out=rc, in_=zt[(g, jj)][:, qc, HD:HD + 1])
                                nc.vector.tensor_scalar_mul(
                                    ztn[g][:, qc, 64 * jj:64 * (jj + 1)],
                                    zt[(g, jj)][:, qc, 0:HD], rc)
                        if ik == nkt - 1:
                            nc.sync.dma_start_transpose(
                                out=z_sb[:, g, jq * QB:(jq + 1) * QB]
                                .rearrange("p (a b) -> p a b", a=4),
                                in_=ztn[g].rearrange("p a b -> p (a b)"))
                    m = ik - 4 * jq
                    off = 128 * m if m > 0 else 0
                    nvalid = 4 - max(0, m)
                    cost = 2 * (QB - off) + 2 * nvalid * (HD + 1)
                    return (cost, f)

                for g in range(2):
                    for ik in range(nkt):
                        items.append(mk_ik(g, ik))
                return items

            def p3_stream(jq):
                # output projection runs in the last window, after all
                # projections: ps_o reuses the freed pp ring (and, for the
                # post-attention jq==3 tail, also the sps ring); drains
                # alternate DVE/Act so neither becomes the cadence limit
                items = []

                osbs = {}

                def mk(it, db, i):
                    def f():
                        if db == 0:
                            osbs[it] = osb.tile([128, 4, 512], BF16, tag="o",
                                                name=f"os{it}")
                        if jq == 3 and i % 2 == 1:
                            ps_o = sps.tile([128, 512], F32, tag="s",
                                            name=f"o{it}_{db}")
                        else:
                            ps_o = pp.tile([128, 512], F32, tag="pp",
                                           name=f"o{it}_{db}")
                        for g in range(2):
                            nc.tensor.matmul(
                                ps_o, z_sb[:, g, it * 128:(it + 1) * 128],
                                wo_sb[:, g, db * 512:(db + 1) * 512],
                                start=(g == 0), stop=(g == 1),
                                skip_group_check=True)
                        if i % 2 == 0:
                            nc.vector.tensor_copy(out=osbs[it][:, db, :],
                                                  in_=ps_o)
                        else:
                            nc.scalar.copy(out=osbs[it][:, db, :], in_=ps_o)
                        if db == 3:
                            nc.sync.dma_start(
                                out=o_d[it * 128:(it + 1) * 128, :],
                                in_=osbs[it].rearrange("p a b -> p (a b)"))
                    return (1024, f)

                i = 0
                for it in range(4 * jq, 4 * jq + 4):
                    for db in range(4):
                        items.append(mk(it, db, i))
                        i += 1
                return items

            def merge(*streams):
                streams = [list(s) for s in streams if s]
                totals = [max(1, sum(c for c, _ in s)) for s in streams]
                done = [0.0] * len(streams)
                idx = [0] * len(streams)
                while True:
                    best, bestf = None, None
                    for i, s in enumerate(streams):
                        if idx[i] < len(s):
                            frac = done[i] / totals[i]
                            if best is None or frac < bestf:
                                best, bestf = i, frac
                    if best is None:
                        break
                    c, f = streams[best][idx[best]]
                    f()
                    done[best] += c
                    idx[best] += 1

            # startup: first x chunk and q/k weights first so PE starts
            # ~6us in; wv/mask/wo are not needed until later
            s0 = proj_stream(0)
            wq_r = wq_d.rearrange("(c p) n -> p c n", p=128)
            wk_r = wk_d.rearrange("(c p) n -> p c n", p=128)
            nc.sync.dma_start(out=wq_sb[:, 0:8, :], in_=wq_r[:, 0:8, :])
            s0[0][1]()  # xtb(0) chunk 0 DMA
            nc.sync.dma_start(out=wq_sb[:, 8:16, :], in_=wq_r[:, 8:16, :])
            nc.sync.dma_start(out=wk_sb[:, 0:8, :], in_=wk_r[:, 0:8, :])
            s0[1][1]()
            nc.sync.dma_start(out=wk_sb[:, 8:16, :], in_=wk_r[:, 8:16, :])
            for c in range(2, 4):
                s0[c][1]()  # xtb(0) chunks 2-3
            nc.sync.dma_start(out=wv_sb, in_=wv_d.rearrange("(c p) n -> p c n", p=128))
            nc.sync.dma_start(out=mask_sb, in_=mask_d)
            nc.sync.dma_start(out=wo_sb, in_=wo_d.rearrange("(g p) d -> p g d", p=128))
            merge(s0[4:])
            merge(att_stream(0), proj_stream(1))
            merge(att_stream(1), proj_stream(2))
            merge(att_stream(2), proj_stream(3))
            # all output projection backfills the Act-bound att(3) window;
            # p3(3) consumes att(3)'s transposes — emitted strictly after
            merge(att_stream(3), p3_stream(0), p3_stream(1), p3_stream(2))
            merge(p3_stream(3))

    nc.compile()
    return nc


def get_program():
    if "nc" not in _CACHE:
        _CACHE["nc"] = _build_program()
    return _CACHE["nc"]


def make_in_maps(normalized_resid_pre, W_Q, W_K, W_V, W_O):
    import ml_dtypes
    bf = ml_dtypes.bfloat16

    x = np.ascontiguousarray(np.asarray(normalized_resid_pre, np.float32))
    W_Q = np.asarray(W_Q, np.float32)
    W_K = np.asarray(W_K, np.float32)
    W_V = np.asarray(W_V, np.float32)
    W_O = np.asarray(W_O, np.float32)
    wqe = W_Q.reshape(NKV, GQ, D, HD).sum(1) * (1.0 / np.sqrt(HD))

    kk = np.arange(128)[:, None]
    qq = np.arange(128)[None, :]
    mask = np.ascontiguousarray(np.where(kk <= qq, 1.0, 0.0).astype(bf))

    in_maps = []
    for c in range(8):
        b, half = divmod(c, 2)
        heads = [4 * half + m for m in range(HPC)]
        xT = np.ascontiguousarray(x[b].T)
        wq = np.ascontiguousarray(np.concatenate([wqe[n] for n in heads], 1))
        wk = np.ascontiguousarray(np.concatenate([W_K[n] for n in heads], 1))
        wv = np.ascontiguousarray(np.concatenate([W_V[n] for n in heads], 1))
        wo = np.ascontiguousarray(np.concatenate([W_O[n] for n in heads], 0))
        in_maps.append({"xT": xT.astype(bf), "wq": wq.astype(bf),
                        "wk": wk.astype(bf), "wv": wv.astype(bf),
                        "wo": wo.astype(bf), "mask": mask})
    return in_maps


def run(in_maps, **kw):
    from concourse.bass_utils import run_bass_kernel_spmd
    return run_bass_kernel_spmd(get_program(), in_maps,
                                core_ids=list(range(8)), **kw)


def kernel(normalized_resid_pre, W_Q, W_K, W_V, W_O):
    in_maps = make_in_maps(normalized_resid_pre, W_Q, W_K, W_V, W_O)
    res = run(in_maps)
    out = np.empty((4, S, D), np.float32)
    for b in range(4):
        out[b] = (res.results[2 * b]["o"].astype(np.float32)
                  + res.results[2 * b + 1]["o"].astype(np.float32))
    return out
